# revision 12
# baseline (speedup 1.0000x reference)
"""Trainium2 Bass kernel for nn_ChemicalDevelopment (drag-scan + separable
Gaussian blur + mask-combine + 3x3 channel coupling + tanh saturation).

Self-contained: hardcodes shapes/sharding. Shards the W (column) axis across
8 NeuronCores; each core processes its full-height column slab independently
(no collectives).

v3 design (fp16, planar channel layout, scaled units x' = x/3):
  - host: deinterleave channels, scale by 1/(3+1e-6) (folds the tanning-mask
    scale and the tanh input scale), compute the 3-tap horizontal blur
    (sigma=0.5 taps beyond +-1 horizontally / +-2 vertically are < 3.4e-4
    relative), ship TWO fp16 streams per core: xs [H, 3*ws], xh [H, 3*ws]
  - PE (fp16 matmuls): causal row scan soft' = T xs + U xs_prev per 128-row
    block; vertical 5-tap blur hard' = B xh + 2-row halo matmuls (32x32
    quadrant-aligned) against neighbour blocks
  - ACT: soft PSUM -> SBUF fp16; final tanh
  - DVE: diff = hard' - soft' (reads PSUM), pp = xs*diff, u = soft' + pp
    (all tensor_tensor, 2x fp16 mode)
  - DVE+POOL: channel mix v_j = xs_j - sum_i C[i,j] u_i (9 thin STTs split
    across both engines)
  - host: out = 3 * tanh result, upcast to f32, reinterleave channels
"""
import numpy as np

H_FULL = 4096
W_FULL = 4096
NCORES = 8
WS = W_FULL // NCORES      # 512 columns per core
P = 128                    # partition block (rows)
HIST = 32                  # scan history rows from previous block
RV = 2                     # vertical blur radius (5 taps)
SIGMA_SOFT = 2.0
SIGMA_HARD = 0.5
D_MAX = 3.0
SINV = 1.0 / (D_MAX + 1e-6)
FC = 3 * WS                # row width (1536)

BATCH = 2                  # row-blocks per elementwise batch group

_NC_CACHE = {}


def _taps25():
    # identical arithmetic to the reference (f32)
    x = np.arange(-12, 13, dtype=np.float32)
    k = np.exp(np.float32(-0.5) * (x / np.float32(SIGMA_HARD)) ** 2)
    k = k / k.sum()
    return k.astype(np.float64)


def _matrices():
    d = np.exp(-1.0 / SIGMA_SOFT)
    scale = 1.0 - d
    i = np.arange(P)[:, None]
    j = np.arange(P)[None, :]
    e = i - j
    with np.errstate(under="ignore"):
        T = np.where(e >= 0, scale * d ** np.clip(e, 0, None), 0.0)
        ih = np.arange(HIST)[:, None]
        jh = np.arange(HIST)[None, :]
        U = scale * d ** (ih + (HIST - jh))   # out row i <- prev row 128-HIST+j
    ky = _taps25()
    k0h = ky[12]
    # vertical 5-tap band, folded with the horizontal center tap k0h
    B = np.where(np.abs(e) <= RV, k0h * ky[np.clip(e + 12, 0, 24)], 0.0)
    # 32x32 halo bands in lhsT orientation (PE quadrant alignment):
    jj = np.arange(32)[:, None]
    ii = np.arange(32)[None, :]
    eu = jj - 32 - ii                      # prev-block row 96+j -> out row i
    upT = np.where(np.abs(eu) <= RV, k0h * ky[np.clip(eu + 12, 0, 24)], 0.0)
    ed = jj + 32 - ii                      # next-block row j -> out row 96+i
    dnT = np.where(np.abs(ed) <= RV, k0h * ky[np.clip(ed + 12, 0, 24)], 0.0)
    f = lambda a: np.ascontiguousarray(a, np.float16)
    return f(T), f(U), f(B), f(upT), f(dnT)


def _build_nc(Hk):
    """SPMD Bass program: xs/xh [Hk, FC] fp16 -> y [Hk, FC] fp16 (tanh(v');
    the x3 and f32 upcast happen on the host)."""
    import concourse.bacc as bacc
    import concourse.mybir as mybir
    from concourse.tile import TileContext

    f16 = mybir.dt.float16
    f32 = mybir.dt.float32
    AO = mybir.AluOpType
    nb = Hk // P

    T, U, B, Bup, Bdn = _matrices()
    wconst_np = np.zeros((128, 384), np.float16)
    wconst_np[:, 0:128] = T.T
    wconst_np[:, 128:256] = B.T
    wconst_np[128 - HIST:128, 256:256 + HIST] = U.T
    wconst_np[96:128, 288:320] = Bup         # upT, contraction rows 96..127
    wconst_np[0:32, 320:352] = Bdn           # dnT, contraction rows 0..31

    nc = bacc.Bacc(trn_type="TRN2", debug=False)
    hxs = nc.dram_tensor("xs", [Hk, FC], f16, kind="ExternalInput")
    hxh = nc.dram_tensor("xh", [Hk, FC], f16, kind="ExternalInput")
    hcm = nc.dram_tensor("cmat", [1, 9], f32, kind="ExternalInput")
    hy = nc.dram_tensor("y", [Hk, FC], f16, kind="ExternalOutput")
    hconst = nc.inline_tensor(wconst_np, name="wconst")

    GF = BATCH * FC            # group tile width (elementwise batch)
    ng = nb // BATCH

    with TileContext(nc) as tc:
        with tc.tile_pool(name="wpool", bufs=1) as wpool, \
             tc.tile_pool(name="cps_pool", bufs=1, space="PSUM") as cpsp, \
             tc.tile_pool(name="xpool", bufs=3) as xpool, \
             tc.tile_pool(name="xhpool", bufs=3) as xhpool, \
             tc.tile_pool(name="sfpool", bufs=2) as sfpool, \
             tc.tile_pool(name="hdpool", bufs=2) as hdpool, \
             tc.tile_pool(name="wk", bufs=2) as wk, \
             tc.tile_pool(name="opool", bufs=2) as opool, \
             tc.tile_pool(name="pss_pool", bufs=1, space="PSUM") as pssp, \
             tc.tile_pool(name="psh_pool", bufs=1, space="PSUM") as pshp:

            wconst = wpool.tile([128, 384], f16, name="wconst_t")
            nc.sync.dma_start(out=wconst, in_=hconst[:, :])
            wT = wconst[:, 0:128]
            wB = wconst[:, 128:256]
            wU = wconst[128 - HIST:128, 256:256 + HIST]
            wBup = wconst[96:128, 288:320]
            wBdn = wconst[0:32, 320:352]

            cmsb = wpool.tile([1, 9], f32, name="cmsb")
            nc.sync.dma_start(out=cmsb, in_=hcm[:, :])
            ones_t = wpool.tile([1, 128], f32, name="ones_t")
            nc.vector.memset(ones_t, 1.0)
            cps = cpsp.tile([128, 16], f32, name="cps")
            nc.tensor.matmul(out=cps[:, 0:9], lhsT=ones_t, rhs=cmsb,
                             start=True, stop=True)
            negc = wpool.tile([128, 16], f32, name="negc")
            nc.scalar.mul(negc[:, 0:9], cps[:, 0:9], -1.0)

            xsB = [None] * ng
            xhB = [None] * ng
            sfB = [None] * ng
            hdB = [None] * ng

            def xs_sl(b):
                g, t = divmod(b, BATCH)
                return xsB[g][:, t * FC:(t + 1) * FC]

            def xh_sl(b):
                g, t = divmod(b, BATCH)
                return xhB[g][:, t * FC:(t + 1) * FC]

            def load(b):
                g, t = divmod(b, BATCH)
                if t == 0:
                    xsB[g] = xpool.tile([128, GF], f16, name=f"xs{g}", tag="xs")
                    xhB[g] = xhpool.tile([128, GF], f16, name=f"xh{g}", tag="xh")
                nc.sync.dma_start(out=xs_sl(b), in_=hxs[b * P:(b + 1) * P, :])
                nc.sync.dma_start(out=xh_sl(b), in_=hxh[b * P:(b + 1) * P, :])

            def process_block(b):
                g, t = divmod(b, BATCH)
                xb = xs_sl(b)
                xhb = xh_sl(b)

                # causal row scan -> ps_s (grouped by weight matrix)
                ps_s = pssp.tile([128, FC], f32, name=f"ps_s{b}", tag="ps_s")
                for p in range(3):
                    nc.tensor.matmul(out=ps_s[:, p * WS:(p + 1) * WS], lhsT=wT,
                                     rhs=xb[:, p * WS:(p + 1) * WS],
                                     start=True, stop=(b == 0))
                if b > 0:
                    xp = xs_sl(b - 1)
                    for p in range(3):
                        nc.tensor.matmul(out=ps_s[0:HIST, p * WS:(p + 1) * WS],
                                         lhsT=wU,
                                         rhs=xp[128 - HIST:128, p * WS:(p + 1) * WS],
                                         start=False, stop=True,
                                         tile_position=(128 - HIST, 0))

                # vertical blur -> ps_h
                ps_h = pshp.tile([128, FC], f32, name=f"ps_h{b}", tag="ps_h")
                for p in range(3):
                    nc.tensor.matmul(out=ps_h[:, p * WS:(p + 1) * WS], lhsT=wB,
                                     rhs=xhb[:, p * WS:(p + 1) * WS],
                                     start=True,
                                     stop=(b == 0 and b == nb - 1))
                if b > 0:
                    xhp = xh_sl(b - 1)
                    for p in range(3):
                        nc.tensor.matmul(out=ps_h[0:32, p * WS:(p + 1) * WS],
                                         lhsT=wBup,
                                         rhs=xhp[96:128, p * WS:(p + 1) * WS],
                                         start=False, stop=(b == nb - 1),
                                         tile_position=(96, 0))
                if b + 1 < nb:
                    xhn = xh_sl(b + 1)
                    for p in range(3):
                        nc.tensor.matmul(out=ps_h[96:128, p * WS:(p + 1) * WS],
                                         lhsT=wBdn,
                                         rhs=xhn[0:32, p * WS:(p + 1) * WS],
                                         start=False, stop=True,
                                         tile_position=(0, 96))

                # PSUM -> SBUF fp16 copies free the banks for the next block
                if t == 0:
                    sfB[g] = sfpool.tile([128, GF], f16, name=f"sf{g}", tag="sf")
                    hdB[g] = hdpool.tile([128, GF], f16, name=f"hd{g}", tag="hd")
                nc.scalar.copy(out=sfB[g][:, t * FC:(t + 1) * FC], in_=ps_s[:, :])
                nc.scalar.copy(out=hdB[g][:, t * FC:(t + 1) * FC], in_=ps_h[:, :])

            def plane(tile, j):
                # [128, BATCH, WS] view of plane j across the group's slots
                return tile.rearrange("p (t cw) -> p t cw", t=BATCH)[
                    :, :, j * WS:(j + 1) * WS]

            uB = [None] * ng

            def phase_a(g):
                # diff = hard - soft ; pp = xs*diff ; u = soft + pp
                soft = sfB[g]
                diff = wk.tile([128, GF], f16, name=f"df{g}", tag="df")
                nc.vector.tensor_tensor(out=diff, in0=hdB[g], in1=soft,
                                        op=AO.subtract)
                pp = wk.tile([128, GF], f16, name=f"pp{g}", tag="pp")
                nc.gpsimd.tensor_tensor(out=pp, in0=xsB[g], in1=diff,
                                        op=AO.mult)
                u = wk.tile([128, GF], f16, name=f"u{g}", tag="u")
                nc.gpsimd.tensor_add(out=u, in0=soft, in1=pp)
                uB[g] = u

            def tail(g):
                # channel mix: v_j = xs_j - sum_i C[i,j] u_i (9 DVE STTs),
                # then tanh (x3 + f32 upcast on host)
                u = uB[g]
                xg = xsB[g]
                v = wk.tile([128, GF], f16, name=f"v{g}", tag="v")
                for j in range(3):
                    for i in range(3):
                        nc.vector.scalar_tensor_tensor(
                            out=plane(v, j), in0=plane(u, i),
                            scalar=negc[:, 3 * i + j:3 * i + j + 1],
                            in1=(plane(xg, j) if i == 0 else plane(v, j)),
                            op0=AO.mult, op1=AO.add)
                ot = opool.tile([128, GF], f16, name=f"o{g}", tag="o")
                nc.scalar.activation(out=ot, in_=v,
                                     func=mybir.ActivationFunctionType.Tanh)
                for t in range(BATCH):
                    b = g * BATCH + t
                    nc.sync.dma_start(out=hy[b * P:(b + 1) * P, :],
                                      in_=ot[:, t * FC:(t + 1) * FC])

            load(0)
            load(1)
            for b in range(nb):
                if b + 2 < nb:
                    load(b + 2)
                process_block(b)
                # software pipeline: phase_a right after the group's blocks,
                # the mix/tanh tail one group later (keeps the in-order DVE
                # queue from stalling on POOL's pp/u)
                if b % BATCH == BATCH - 1:
                    g = b // BATCH
                    phase_a(g)
                    if g >= 1:
                        tail(g - 1)
            tail(ng - 1)

    nc.finalize()
    return nc


def _get_nc(Hk):
    if Hk not in _NC_CACHE:
        _NC_CACHE[Hk] = _build_nc(Hk)
    return _NC_CACHE[Hk]


def prepare(D_macro, coupling_matrix):
    D = np.asarray(D_macro, dtype=np.float32)
    C = np.ascontiguousarray(np.asarray(coupling_matrix, np.float32).reshape(1, 9))
    Hk, Wk, _ = D.shape
    ws = Wk // NCORES
    ky = _taps25()
    r1 = np.float32(ky[13] / ky[12])
    # planar scaled units: xs = x/3 (H, 3, W); horizontal 3-tap blur on host
    xs = np.transpose(D, (0, 2, 1)) * np.float32(SINV)
    xp = np.pad(xs, ((0, 0), (0, 0), (1, 1)))
    xh = xs + r1 * (xp[:, :, 0:-2] + xp[:, :, 2:])
    xs16 = xs.astype(np.float16)
    xh16 = xh.astype(np.float16)
    in_maps = []
    for m in range(NCORES):
        sl_s = np.ascontiguousarray(
            xs16[:, :, m * ws:(m + 1) * ws]).reshape(Hk, 3 * ws)
        sl_h = np.ascontiguousarray(
            xh16[:, :, m * ws:(m + 1) * ws]).reshape(Hk, 3 * ws)
        in_maps.append({"xs": sl_s, "xh": sl_h, "cmat": C})
    nc = _get_nc(Hk)
    return in_maps, nc


def kernel(D_macro, coupling_matrix):
    from concourse.bass_utils import run_bass_kernel_spmd

    Hk, Wk, _ = np.asarray(D_macro).shape
    ws = Wk // NCORES
    in_maps, nc = prepare(D_macro, coupling_matrix)
    res = run_bass_kernel_spmd(nc, in_maps, core_ids=list(range(NCORES)))
    out = np.empty((Hk, Wk, 3), np.float32)
    for m, r in enumerate(res.results):
        y = r["y"].reshape(Hk, 3, ws)          # planar fp16
        out[:, m * ws:(m + 1) * ws, :] = np.transpose(y, (0, 2, 1))
    np.multiply(out, np.float32(D_MAX), out=out)
    return out


# revision 13
# speedup vs baseline: 1.0619x; 1.0619x over previous
"""Trainium2 Bass kernel for nn_ChemicalDevelopment (drag-scan + separable
Gaussian blur + mask-combine + 3x3 channel coupling + tanh saturation).

Self-contained: hardcodes shapes/sharding. Shards the W (column) axis across
8 NeuronCores; each core processes its full-height column slab independently
(no collectives).

v3 design (fp16, planar channel layout, scaled units x' = x/3):
  - host: deinterleave channels, scale by 1/(3+1e-6) (folds the tanning-mask
    scale and the tanh input scale), compute the 3-tap horizontal blur
    (sigma=0.5 taps beyond +-1 horizontally / +-2 vertically are < 3.4e-4
    relative), ship TWO fp16 streams per core: xs [H, 3*ws], xh [H, 3*ws]
  - PE (fp16 matmuls): causal row scan soft' = T xs + U xs_prev per 128-row
    block; vertical 5-tap blur hard' = B xh + 2-row halo matmuls (32x32
    quadrant-aligned) against neighbour blocks
  - ACT: soft PSUM -> SBUF fp16; final tanh
  - DVE: diff = hard' - soft' (reads PSUM), pp = xs*diff, u = soft' + pp
    (all tensor_tensor, 2x fp16 mode)
  - DVE+POOL: channel mix v_j = xs_j - sum_i C[i,j] u_i (9 thin STTs split
    across both engines)
  - host: out = 3 * tanh result, upcast to f32, reinterleave channels
"""
import numpy as np

H_FULL = 4096
W_FULL = 4096
NCORES = 8
WS = W_FULL // NCORES      # 512 columns per core
P = 128                    # partition block (rows)
HIST = 32                  # scan history rows from previous block
RV = 2                     # vertical blur radius (5 taps)
SIGMA_SOFT = 2.0
SIGMA_HARD = 0.5
D_MAX = 3.0
SINV = 1.0 / (D_MAX + 1e-6)
FC = 3 * WS                # row width (1536)

BATCH = 2                  # row-blocks per elementwise batch group

_NC_CACHE = {}


def _taps25():
    # identical arithmetic to the reference (f32)
    x = np.arange(-12, 13, dtype=np.float32)
    k = np.exp(np.float32(-0.5) * (x / np.float32(SIGMA_HARD)) ** 2)
    k = k / k.sum()
    return k.astype(np.float64)


def _matrices():
    d = np.exp(-1.0 / SIGMA_SOFT)
    scale = 1.0 - d
    i = np.arange(P)[:, None]
    j = np.arange(P)[None, :]
    e = i - j
    with np.errstate(under="ignore"):
        T = np.where(e >= 0, scale * d ** np.clip(e, 0, None), 0.0)
        ih = np.arange(HIST)[:, None]
        jh = np.arange(HIST)[None, :]
        U = scale * d ** (ih + (HIST - jh))   # out row i <- prev row 128-HIST+j
    ky = _taps25()
    k0h = ky[12]
    # vertical 5-tap band, folded with the horizontal center tap k0h
    B = np.where(np.abs(e) <= RV, k0h * ky[np.clip(e + 12, 0, 24)], 0.0)
    # 32x32 halo bands in lhsT orientation (PE quadrant alignment):
    jj = np.arange(32)[:, None]
    ii = np.arange(32)[None, :]
    eu = jj - 32 - ii                      # prev-block row 96+j -> out row i
    upT = np.where(np.abs(eu) <= RV, k0h * ky[np.clip(eu + 12, 0, 24)], 0.0)
    ed = jj + 32 - ii                      # next-block row j -> out row 96+i
    dnT = np.where(np.abs(ed) <= RV, k0h * ky[np.clip(ed + 12, 0, 24)], 0.0)
    f = lambda a: np.ascontiguousarray(a, np.float16)
    return f(T), f(U), f(B), f(upT), f(dnT)


def _build_nc(Hk):
    """SPMD Bass program: xs/xh [Hk, FC] fp16 -> y [Hk, FC] fp16 (tanh(v');
    the x3 and f32 upcast happen on the host)."""
    import concourse.bacc as bacc
    import concourse.mybir as mybir
    from concourse.tile import TileContext

    f16 = mybir.dt.float16
    f32 = mybir.dt.float32
    AO = mybir.AluOpType
    nb = Hk // P

    T, U, B, Bup, Bdn = _matrices()
    wconst_np = np.zeros((128, 384), np.float16)
    wconst_np[:, 0:128] = T.T
    wconst_np[:, 128:256] = B.T
    wconst_np[128 - HIST:128, 256:256 + HIST] = U.T
    wconst_np[96:128, 288:320] = Bup         # upT, contraction rows 96..127
    wconst_np[0:32, 320:352] = Bdn           # dnT, contraction rows 0..31

    nc = bacc.Bacc(trn_type="TRN2", debug=False)
    hxs = nc.dram_tensor("xs", [Hk, FC], f16, kind="ExternalInput")
    hxh = nc.dram_tensor("xh", [Hk, FC], f16, kind="ExternalInput")
    hcm = nc.dram_tensor("cmat", [1, 9], f32, kind="ExternalInput")
    hy = nc.dram_tensor("y", [Hk, FC], f16, kind="ExternalOutput")
    hconst = nc.inline_tensor(wconst_np, name="wconst")

    GF = BATCH * FC            # group tile width (elementwise batch)
    ng = nb // BATCH

    with TileContext(nc) as tc:
        with tc.tile_pool(name="wpool", bufs=1) as wpool, \
             tc.tile_pool(name="cps_pool", bufs=1, space="PSUM") as cpsp, \
             tc.tile_pool(name="xpool", bufs=3) as xpool, \
             tc.tile_pool(name="xhpool", bufs=3) as xhpool, \
             tc.tile_pool(name="sfpool", bufs=2) as sfpool, \
             tc.tile_pool(name="hdpool", bufs=2) as hdpool, \
             tc.tile_pool(name="wk", bufs=2) as wk, \
             tc.tile_pool(name="opool", bufs=2) as opool, \
             tc.tile_pool(name="pss_pool", bufs=1, space="PSUM") as pssp, \
             tc.tile_pool(name="psh_pool", bufs=1, space="PSUM") as pshp:

            wconst = wpool.tile([128, 384], f16, name="wconst_t")
            nc.sync.dma_start(out=wconst, in_=hconst[:, :])
            wT = wconst[:, 0:128]
            wB = wconst[:, 128:256]
            wU = wconst[128 - HIST:128, 256:256 + HIST]
            wBup = wconst[96:128, 288:320]
            wBdn = wconst[0:32, 320:352]

            cmsb = wpool.tile([1, 9], f32, name="cmsb")
            nc.sync.dma_start(out=cmsb, in_=hcm[:, :])
            ones_t = wpool.tile([1, 128], f32, name="ones_t")
            nc.vector.memset(ones_t, 1.0)
            cps = cpsp.tile([128, 16], f32, name="cps")
            nc.tensor.matmul(out=cps[:, 0:9], lhsT=ones_t, rhs=cmsb,
                             start=True, stop=True)
            negc = wpool.tile([128, 16], f32, name="negc")
            nc.scalar.mul(negc[:, 0:9], cps[:, 0:9], -1.0)

            xsB = [None] * ng
            xhB = [None] * ng
            sfB = [None] * ng
            hdB = [None] * ng

            def xs_sl(b):
                g, t = divmod(b, BATCH)
                return xsB[g][:, t * FC:(t + 1) * FC]

            def xh_sl(b):
                g, t = divmod(b, BATCH)
                return xhB[g][:, t * FC:(t + 1) * FC]

            def load(b):
                g, t = divmod(b, BATCH)
                if t == 0:
                    xsB[g] = xpool.tile([128, GF], f16, name=f"xs{g}", tag="xs")
                    xhB[g] = xhpool.tile([128, GF], f16, name=f"xh{g}", tag="xh")
                nc.sync.dma_start(out=xs_sl(b), in_=hxs[b * P:(b + 1) * P, :])
                nc.sync.dma_start(out=xh_sl(b), in_=hxh[b * P:(b + 1) * P, :])

            def process_block(b):
                g, t = divmod(b, BATCH)
                xb = xs_sl(b)
                xhb = xh_sl(b)

                # causal row scan -> ps_s (grouped by weight matrix)
                ps_s = pssp.tile([128, FC], f32, name=f"ps_s{b}", tag="ps_s")
                for p in range(3):
                    nc.tensor.matmul(out=ps_s[:, p * WS:(p + 1) * WS], lhsT=wT,
                                     rhs=xb[:, p * WS:(p + 1) * WS],
                                     start=True, stop=(b == 0))
                if b > 0:
                    xp = xs_sl(b - 1)
                    for p in range(3):
                        nc.tensor.matmul(out=ps_s[0:HIST, p * WS:(p + 1) * WS],
                                         lhsT=wU,
                                         rhs=xp[128 - HIST:128, p * WS:(p + 1) * WS],
                                         start=False, stop=True,
                                         tile_position=(128 - HIST, 0))

                # vertical blur -> ps_h
                ps_h = pshp.tile([128, FC], f32, name=f"ps_h{b}", tag="ps_h")
                for p in range(3):
                    nc.tensor.matmul(out=ps_h[:, p * WS:(p + 1) * WS], lhsT=wB,
                                     rhs=xhb[:, p * WS:(p + 1) * WS],
                                     start=True,
                                     stop=(b == 0 and b == nb - 1))
                if b > 0:
                    xhp = xh_sl(b - 1)
                    for p in range(3):
                        nc.tensor.matmul(out=ps_h[0:32, p * WS:(p + 1) * WS],
                                         lhsT=wBup,
                                         rhs=xhp[96:128, p * WS:(p + 1) * WS],
                                         start=False, stop=(b == nb - 1),
                                         tile_position=(96, 0))
                if b + 1 < nb:
                    xhn = xh_sl(b + 1)
                    for p in range(3):
                        nc.tensor.matmul(out=ps_h[96:128, p * WS:(p + 1) * WS],
                                         lhsT=wBdn,
                                         rhs=xhn[0:32, p * WS:(p + 1) * WS],
                                         start=False, stop=True,
                                         tile_position=(0, 96))

                # PSUM -> SBUF fp16 copies free the banks for the next block
                if t == 0:
                    sfB[g] = sfpool.tile([128, GF], f16, name=f"sf{g}", tag="sf")
                    hdB[g] = hdpool.tile([128, GF], f16, name=f"hd{g}", tag="hd")
                nc.scalar.copy(out=sfB[g][:, t * FC:(t + 1) * FC], in_=ps_s[:, :])
                nc.scalar.copy(out=hdB[g][:, t * FC:(t + 1) * FC], in_=ps_h[:, :])

            def plane(tile, j):
                # [128, BATCH, WS] view of plane j across the group's slots
                return tile.rearrange("p (t cw) -> p t cw", t=BATCH)[
                    :, :, j * WS:(j + 1) * WS]

            def process_group(g):
                soft = sfB[g]
                hard = hdB[g]
                xg = xsB[g]
                # diff = hard - soft ; pp = xs*diff ; u = soft + pp
                diff = wk.tile([128, GF], f16, name=f"df{g}", tag="df")
                nc.vector.tensor_tensor(out=diff, in0=hard, in1=soft,
                                        op=AO.subtract)
                pp = wk.tile([128, GF], f16, name=f"pp{g}", tag="pp")
                nc.gpsimd.tensor_tensor(out=pp, in0=xg, in1=diff, op=AO.mult)
                u = wk.tile([128, GF], f16, name=f"u{g}", tag="u")
                nc.gpsimd.tensor_add(out=u, in0=soft, in1=pp)

                # channel mix: v_j = xs_j - sum_i C[i,j] u_i
                # j = 0,1 as STT chains on DVE; j = 2 as ACT muls + DVE adds
                v = wk.tile([128, GF], f16, name=f"v{g}", tag="v")
                pt = [wk.tile([128, BATCH * WS], f16, name=f"pt{i}_{g}",
                              tag=f"pt{i}") for i in range(3)]
                for i in range(3):
                    nc.scalar.mul(pt[i].rearrange("p (t w) -> p t w", t=BATCH),
                                  plane(u, i), negc[:, 3 * i + 2:3 * i + 3])
                for j in range(2):
                    for i in range(3):
                        nc.vector.scalar_tensor_tensor(
                            out=plane(v, j), in0=plane(u, i),
                            scalar=negc[:, 3 * i + j:3 * i + j + 1],
                            in1=(plane(xg, j) if i == 0 else plane(v, j)),
                            op0=AO.mult, op1=AO.add)
                acc = plane(v, 2)
                nc.vector.tensor_add(
                    out=acc, in0=plane(xg, 2),
                    in1=pt[0].rearrange("p (t w) -> p t w", t=BATCH))
                nc.vector.tensor_add(
                    out=acc, in0=acc,
                    in1=pt[1].rearrange("p (t w) -> p t w", t=BATCH))
                nc.vector.tensor_add(
                    out=acc, in0=acc,
                    in1=pt[2].rearrange("p (t w) -> p t w", t=BATCH))

                # out = tanh(v)  (x3 + f32 upcast on host)
                ot = opool.tile([128, GF], f16, name=f"o{g}", tag="o")
                nc.scalar.activation(out=ot, in_=v,
                                     func=mybir.ActivationFunctionType.Tanh)
                for t in range(BATCH):
                    b = g * BATCH + t
                    nc.sync.dma_start(out=hy[b * P:(b + 1) * P, :],
                                      in_=ot[:, t * FC:(t + 1) * FC])

            load(0)
            load(1)
            for b in range(nb):
                if b + 2 < nb:
                    load(b + 2)
                process_block(b)
                # lag group elementwise by one block so the next block's
                # PSUM-freeing ACT copies aren't stuck behind group ACT ops
                if b >= 1 and (b - 1) % BATCH == BATCH - 1:
                    process_group((b - 1) // BATCH)
            process_group(ng - 1)

    nc.finalize()
    return nc


def _get_nc(Hk):
    if Hk not in _NC_CACHE:
        _NC_CACHE[Hk] = _build_nc(Hk)
    return _NC_CACHE[Hk]


def prepare(D_macro, coupling_matrix):
    D = np.asarray(D_macro, dtype=np.float32)
    C = np.ascontiguousarray(np.asarray(coupling_matrix, np.float32).reshape(1, 9))
    Hk, Wk, _ = D.shape
    ws = Wk // NCORES
    ky = _taps25()
    r1 = np.float32(ky[13] / ky[12])
    # planar scaled units: xs = x/3 (H, 3, W); horizontal 3-tap blur on host
    xs = np.transpose(D, (0, 2, 1)) * np.float32(SINV)
    xp = np.pad(xs, ((0, 0), (0, 0), (1, 1)))
    xh = xs + r1 * (xp[:, :, 0:-2] + xp[:, :, 2:])
    xs16 = xs.astype(np.float16)
    xh16 = xh.astype(np.float16)
    in_maps = []
    for m in range(NCORES):
        sl_s = np.ascontiguousarray(
            xs16[:, :, m * ws:(m + 1) * ws]).reshape(Hk, 3 * ws)
        sl_h = np.ascontiguousarray(
            xh16[:, :, m * ws:(m + 1) * ws]).reshape(Hk, 3 * ws)
        in_maps.append({"xs": sl_s, "xh": sl_h, "cmat": C})
    nc = _get_nc(Hk)
    return in_maps, nc


def kernel(D_macro, coupling_matrix):
    from concourse.bass_utils import run_bass_kernel_spmd

    Hk, Wk, _ = np.asarray(D_macro).shape
    ws = Wk // NCORES
    in_maps, nc = prepare(D_macro, coupling_matrix)
    res = run_bass_kernel_spmd(nc, in_maps, core_ids=list(range(NCORES)))
    out = np.empty((Hk, Wk, 3), np.float32)
    for m, r in enumerate(res.results):
        y = r["y"].reshape(Hk, 3, ws)          # planar fp16
        out[:, m * ws:(m + 1) * ws, :] = np.transpose(y, (0, 2, 1))
    np.multiply(out, np.float32(D_MAX), out=out)
    return out
